# revision 12
# baseline (speedup 1.0000x reference)
"""Self-contained Trainium2 (Bass/Tile) kernel for segment-softmax GNN
attention (nn_Attention_6047313953470).

    out[r] = exp(e_r) / sum_{r': idx[r']=idx[r]} exp(e_r')
    e_r = leaky_relu(dot(cat(x_i[r], x_j[r]), a[head(r)]), 0.2)

(The reference subtracts a per-segment max before exp; softmax is invariant
to that shift, and with these magnitudes exp() cannot overflow in f32, so the
shift is dropped. The reference's +1e-16 denominator term is negligible
because every segment sum is >= exp(leaky(min_score)) >> pad contributions.)

Strategy (segments device-local; no collectives):
- Host packs edges sorted by destination segment. Each segment padded to a
  multiple of 8 ("groups" of slots). Segments spanning k groups form class
  k; a class-k chunk holds floor(64/k) such segments (512 slots = 64
  groups), so the ragged segment structure becomes fixed-stride windows:
    matmul (block-diag a) -> 4-head scores -> mask-select -> leaky+exp ->
    group sums (strided reduce) -> k-window segment sums -> reciprocal ->
    broadcast-multiply -> out.
- Head selection ships as ONE byte per (chunk,slot): a [32,512] head-id
  block per round. On device a tiny replicate-matmul fans chunk-rows out to
  the 128 score rows, and the scalar engine forms the one-hot mask as
  relu(1 - (hid - row_head)^2) — exact for integer ids.
- Pad slots are fake head-0 edges with poison features (a_0 . x_pad = -30),
  so exp(-30) vanishes against every real segment sum.
- Chunks are dealt round-robin to the 8 cores so every core has the same
  per-class counts (the SPMD kernel is identical on all cores).
- Features + head-ids stream fp8 (e3m4); weights fp16; output f16. Host
  scatters the packed output back to edge order.
"""
import sys

sys.path.insert(0, "/opt/trn_rl_repo")

from contextlib import ExitStack

import ml_dtypes
import numpy as np

G = 8
CHUNK = 512
NG = CHUNK // G
NCORES = 8
ROUND_PAIRS = 16                      # pair-columns per full round
FEAT_COLS = ROUND_PAIRS * CHUNK       # feature cols per full round
FP16 = np.float16
FP8 = ml_dtypes.float8_e3m4
PAD_SCORE = -30.0
USE_LRELU = False    # HW Lrelu gives wrong results and thrashes ACT tables

_NC_CACHE = {}


def _round_geometry(nchunk):
    npairs = nchunk // 2
    nfull = npairs // ROUND_PAIRS
    rem = npairs % ROUND_PAIRS
    round_pairs = [ROUND_PAIRS] * nfull + ([rem] if rem else [])
    # blocks of 4 rounds; the last rounds run in blocks of 2 then 1 so the
    # end-of-stream drain chain (exp -> window sums -> normalize -> store)
    # covers as few chunks as possible
    blocks = []
    r = 0
    n = len(round_pairs)
    while r < n:
        left = n - r
        take = 4 if left > 6 else (2 if left > 2 else 1)
        take = min(take, left)
        blocks.append(list(range(r, r + take)))
        r += take
    # stream layout: per block [hid 512 cols] + per-round feature cols
    hid_off = {}
    feat_off = {}
    off = 0
    for b, rounds in enumerate(blocks):
        hid_off[b] = off
        off += CHUNK
        for r in rounds:
            feat_off[r] = off
            off += round_pairs[r] * CHUNK
    return round_pairs, blocks, hid_off, feat_off, off


# --------------------------------------------------------------------------
# host-side packing
# --------------------------------------------------------------------------

def _pack(x_i, x_j, a, edge_index, num_nodes):
    HE, D = x_i.shape
    heads = a.shape[0]
    E = HE // heads
    idx = np.asarray(edge_index[1], dtype=np.int64)

    order = np.argsort(idx, kind="stable")
    sidx = idx[order]
    uniq, starts, counts = np.unique(sidx, return_index=True,
                                     return_counts=True)
    nseg = len(uniq)
    ngroups = (counts + G - 1) // G
    if ngroups.max() > NG:
        raise ValueError(f"segment too large: {counts.max()}")

    # ---- class assignment: segments with k groups -> class-k chunks ----
    kk = ngroups.copy()
    while True:
        ks = sorted(set(int(k) for k in kk if k < NG))
        for k in ks:
            n = int((kk == k).sum())
            m = NG // k
            if (n + m - 1) // m < 2 * NCORES:
                kk[kk == k] += 1
                break
        else:
            break
    classes = sorted(set(int(k) for k in kk))
    seg_order = {k: np.nonzero(kk == k)[0] for k in classes}
    mc = {}       # per-core chunk count for class k
    for k in classes:
        m = NG // k
        ck = (len(seg_order[k]) + m - 1) // m
        ck = ((ck + NCORES - 1) // NCORES) * NCORES
        mc[k] = ck // NCORES
    nchunk_core = sum(mc.values())
    if nchunk_core % 2:
        nchunk_core += 1
        mc[classes[-1]] += 1

    runs = []
    off = 0
    for k in classes:
        runs.append((off, off + mc[k], k))
        off += mc[k]

    class_off = {}
    off = 0
    for k in classes:
        class_off[k] = off
        off += mc[k]

    # ---- segment -> (chunk, slot range) ----
    chunk_of_seg = np.empty(nseg, dtype=np.int64)
    gstart_of_seg = np.empty(nseg, dtype=np.int64)
    for k in classes:
        segs = seg_order[k]
        m = NG // k
        ci = np.arange(len(segs)) // m
        core = ci % NCORES
        j = ci // NCORES
        chunk_of_seg[segs] = core * nchunk_core + class_off[k] + j
        gstart_of_seg[segs] = (np.arange(len(segs)) % m) * k

    seg_of_sorted = np.repeat(np.arange(nseg), counts)
    pos_in_seg = np.arange(HE) - np.repeat(starts, counts)
    slot_flat = (chunk_of_seg[seg_of_sorted] * CHUNK
                 + gstart_of_seg[seg_of_sorted] * G + pos_in_seg)
    slot_of_edge = np.empty(HE, dtype=np.int64)
    slot_of_edge[order] = slot_flat

    # ---- poison features: pads are fake head-0 edges, a_0 . x_pad = -30
    a_cat = np.asarray(a, dtype=np.float32)[:, 0, :]      # [4, 64]
    a0 = a_cat[0]
    x_pad = (PAD_SCORE / float(a0 @ a0)) * a0
    x_pad8 = x_pad.astype(FP8).astype(np.float32)
    assert abs(float(a0 @ x_pad8) - PAD_SCORE) < 3.0

    total_slots = NCORES * nchunk_core * CHUNK
    feat = np.empty((total_slots, 2 * D), dtype=np.float32)
    feat[:] = x_pad8[None, :]
    feat[slot_flat, :D] = np.asarray(x_i, dtype=np.float32)[order]
    feat[slot_flat, D:] = np.asarray(x_j, dtype=np.float32)[order]
    head_of_edge = np.arange(HE) // E
    head_at_slot = np.zeros(total_slots, dtype=np.int64)   # pads: head 0
    head_at_slot[slot_flat] = head_of_edge[order]

    round_pairs, blocks, hid_off, feat_off, stream_cols = \
        _round_geometry(nchunk_core)
    npairs_core = nchunk_core // 2

    per_core = {}
    for i in range(NCORES):
        c0, c1 = i * nchunk_core, (i + 1) * nchunk_core
        f = feat[c0 * CHUNK:c1 * CHUNK].reshape(nchunk_core, CHUNK, 2 * D)
        fT = f.transpose(2, 0, 1).reshape(2 * D, npairs_core, 2, CHUNK)
        featT128 = fT.transpose(2, 0, 1, 3).reshape(4 * D,
                                                    npairs_core * CHUNK)
        hid_core = head_at_slot[c0 * CHUNK:c1 * CHUNK].reshape(nchunk_core,
                                                               CHUNK)
        stream = np.zeros((128, stream_cols), dtype=np.float32)
        p0 = 0
        for b, rounds in enumerate(blocks):
            hb = np.full((128, CHUNK), 7.0, dtype=np.float32)
            for u, r in enumerate(rounds):
                npr = round_pairs[r]
                stream[:, feat_off[r]:feat_off[r] + npr * CHUNK] = \
                    featT128[:, p0 * CHUNK:(p0 + npr) * CHUNK]
                hb[32 * u:32 * u + 2 * npr] = hid_core[2 * p0:
                                                       2 * (p0 + npr)]
                p0 += npr
            stream[:, hid_off[b]:hid_off[b] + CHUNK] = hb
        per_core[i] = dict(stream=np.ascontiguousarray(stream).astype(FP8))

    lhs8 = np.zeros((128, 8), dtype=np.float32)
    lhs8[:64, 0:4] = a_cat.T
    lhs8[64:, 4:8] = a_cat.T
    lhs32 = np.zeros((128, 4, 32), dtype=np.float32)
    for j in range(4):
        lhs32[:, j, 8 * j:8 * j + 8] = lhs8
    consts = np.zeros((128, 289), dtype=np.float32)
    consts[:, :128] = lhs32.reshape(128, 128)
    for cc in range(32):
        consts[4 * cc:4 * cc + 4, 128 + cc] = 1.0
    p = np.arange(128)
    # replicate matrix R: consts[p, 160+j] = 1 iff p%32 == j//4
    consts[:, 160:288] = (p[:, None] % 32 == (np.arange(128)[None, :] // 4))
    consts[:, 288] = -(p % 4)                     # bias for (hid - head)^2
    consts = consts.astype(FP16)

    meta = dict(nchunk_core=nchunk_core, slot_of_edge=slot_of_edge,
                runs=tuple(runs), consts=consts)
    return per_core, meta


# --------------------------------------------------------------------------
# device kernel
# --------------------------------------------------------------------------

def _build_nc(nchunk, runs):
    import concourse.tile as tile
    from concourse import bacc, mybir
    from concourse._compat import with_exitstack

    F32 = mybir.dt.float32
    F16 = mybir.dt.float16
    F8 = mybir.dt.float8e3

    @with_exitstack
    def build_kernel(ctx: ExitStack, tc):
        nc = tc.nc
        round_pairs, blocks, hid_off, feat_off, stream_cols = \
            _round_geometry(nchunk)

        stream_d = nc.dram_tensor("stream", [128, stream_cols], F8,
                                  kind="ExternalInput").ap()
        consts_d = nc.dram_tensor("consts", [128, 289], F16,
                                  kind="ExternalInput").ap()
        out_d = nc.dram_tensor("out", [nchunk, CHUNK], F16,
                               kind="ExternalOutput").ap()

        const_pool = ctx.enter_context(tc.tile_pool(name="consts", bufs=1))
        feat_pool = ctx.enter_context(tc.tile_pool(name="feat", bufs=9))
        hid_pool = ctx.enter_context(tc.tile_pool(name="hid", bufs=3))
        eq_pool = ctx.enter_context(tc.tile_pool(name="eq", bufs=3))
        msked_pool = ctx.enter_context(tc.tile_pool(name="msked", bufs=3))
        p_pool = ctx.enter_context(tc.tile_pool(name="p", bufs=2))
        small_pool = ctx.enter_context(tc.tile_pool(name="small", bufs=2))
        out_pool = ctx.enter_context(tc.tile_pool(name="out", bufs=2))
        psum1_pool = ctx.enter_context(tc.tile_pool(name="ps1", bufs=3,
                                                    space="PSUM"))
        psum2_pool = ctx.enter_context(tc.tile_pool(name="ps2", bufs=2,
                                                    space="PSUM"))
        psumh_pool = ctx.enter_context(tc.tile_pool(name="psh", bufs=2,
                                                    space="PSUM"))

        # consts + head-id blocks go on the Scalar HWDGE queue and output
        # stores on the GpSimd SWDGE queue so none of them queue behind the
        # multi-MB feature stream on the Sync HWDGE queue.
        consts = const_pool.tile([128, 289], F16)
        nc.scalar.dma_start(consts[:], consts_d)

        ft_tiles = {}
        for r in blocks[0][:2]:
            npr = round_pairs[r]
            ft = feat_pool.tile([128, FEAT_COLS], F8, tag="feat")
            nc.sync.dma_start(
                ft[:, :npr * CHUNK],
                stream_d[:, feat_off[r]:feat_off[r] + npr * CHUNK])
            ft_tiles[r] = ft
        lhs32 = consts[:, :128]
        ones32 = consts[:, 128:160]
        repl = consts[:, 160:288]
        neg_head = consts[:, 288:289]

        bc0 = 0
        for b, rounds in enumerate(blocks):
            bsz = sum(2 * round_pairs[r] for r in rounds)
            hidt = hid_pool.tile([128, CHUNK], F8, tag="hid")
            nc.scalar.dma_start(hidt[:],
                                stream_d[:, hid_off[b]:hid_off[b] + CHUNK])
            ps2 = psum2_pool.tile([128, CHUNK], F32, space="PSUM")
            p_t = p_pool.tile([128, CHUNK], F32)
            for u, r in enumerate(rounds):
                npr = round_pairs[r]
                if r in ft_tiles:
                    ft = ft_tiles.pop(r)
                else:
                    ft = feat_pool.tile([128, FEAT_COLS], F8, tag="feat")
                    nc.sync.dma_start(
                        ft[:, :npr * CHUNK],
                        stream_d[:, feat_off[r]:feat_off[r] + npr * CHUNK])

                # head-id -> one-hot mask: replicate chunk rows x4 via PE,
                # then relu(1 - (hid - row_head)^2) on the scalar engine
                hidp = psumh_pool.tile([128, CHUNK], F32, space="PSUM")
                nc.tensor.matmul(
                    out=hidp[:],
                    lhsT=repl[32 * u:32 * u + 32, :],
                    rhs=hidt[32 * u:32 * u + 32, :],
                    start=True, stop=True,
                    tile_position=(32 * u, 0),
                )
                sqt = eq_pool.tile([128, CHUNK], F32, tag="sq")
                nc.scalar.activation(sqt[:], hidp[:],
                                     mybir.ActivationFunctionType.Square,
                                     bias=neg_head)
                eqm = eq_pool.tile([128, CHUNK], F16, tag="eq")
                nc.scalar.activation(eqm[:], sqt[:],
                                     mybir.ActivationFunctionType.Relu,
                                     scale=-1.0, bias=1.0)

                ps1 = psum1_pool.tile([128, CHUNK], F32, space="PSUM")
                for k in range(npr):
                    q, j = divmod(k, 4)
                    nc.tensor.matmul(
                        out=ps1[32 * q:32 * (q + 1), :],
                        lhsT=lhs32[:, 32 * j:32 * (j + 1)],
                        rhs=ft[:, k * CHUNK:(k + 1) * CHUNK],
                        start=(j == 0), stop=(j == 3 or k == npr - 1),
                        tile_position=(0, 32 * q),
                    )
                msked = msked_pool.tile([128, CHUNK], F16, tag="msked")
                nc.vector.tensor_tensor(out=msked[:], in0=ps1[:],
                                        in1=eqm[:],
                                        op=mybir.AluOpType.mult)
                nc.tensor.matmul(
                    out=ps2[32 * u:32 * u + 2 * npr, :],
                    lhsT=ones32[:, :2 * npr],
                    rhs=msked[:],
                    start=True, stop=True,
                    tile_position=(0, 32 * u),
                )

            # p = exp(leaky_relu(score, 0.2)), both on the scalar engine
            if USE_LRELU:
                et = p_pool.tile([128, CHUNK], F32, tag="et")
                nc.scalar.activation(et[:bsz, :], ps2[:bsz, :],
                                     mybir.ActivationFunctionType.Lrelu,
                                     alpha=0.2)
            else:
                sx = p_pool.tile([128, CHUNK], F32, tag="sx")
                nc.scalar.activation(sx[:bsz, :], ps2[:bsz, :],
                                     mybir.ActivationFunctionType.Copy,
                                     scale=0.2)
                et = p_pool.tile([128, CHUNK], F32, tag="et")
                nc.vector.tensor_tensor(out=et[:bsz, :], in0=ps2[:bsz, :],
                                        in1=sx[:bsz, :],
                                        op=mybir.AluOpType.max)
            nc.scalar.activation(p_t[:bsz, :], et[:bsz, :],
                                 mybir.ActivationFunctionType.Exp)

            gs = small_pool.tile([128, NG], F32, tag="gs")
            nc.vector.tensor_reduce(
                out=gs[:bsz, :],
                in_=p_t[:bsz, :].rearrange("p (g e) -> p g e", e=G),
                axis=mybir.AxisListType.X, op=mybir.AluOpType.add)

            for (c0, c1, k) in runs:
                r0, r1 = max(c0 - bc0, 0), min(c1 - bc0, bsz)
                if r0 >= r1:
                    continue
                m = NG // k
                w = k * G
                a0, a1 = (r0 // 32) * 32, ((r1 + 31) // 32) * 32
                if a0 == 32 and a1 > 64:
                    spans = [(32, 64), (64, a1)]
                else:
                    spans = [(a0, a1)]
                segsum = small_pool.tile([128, NG], F32, tag=f"ss{k}")
                invS = small_pool.tile([128, NG], F32, tag=f"inv{k}")
                ot = out_pool.tile([128, CHUNK], F16, tag=f"ot{k}")
                for (s0, s1) in spans:
                    rows = s1 - s0
                    nc.vector.tensor_reduce(
                        out=segsum[s0:s1, :m],
                        in_=gs[s0:s1, :m * k].rearrange("p (s k) -> p s k",
                                                        k=k),
                        axis=mybir.AxisListType.X, op=mybir.AluOpType.add)
                    nc.vector.reciprocal(out=invS[s0:s1, :m],
                                         in_=segsum[s0:s1, :m])
                    nc.vector.tensor_tensor(
                        out=ot[s0:s1, :m * w].rearrange("p (s w) -> p s w",
                                                        w=w),
                        in0=p_t[s0:s1, :m * w].rearrange("p (s w) -> p s w",
                                                         w=w),
                        in1=invS[s0:s1, :m].unsqueeze(2).to_broadcast(
                            [rows, m, w]),
                        op=mybir.AluOpType.mult)
                nc.gpsimd.dma_start(out_d[bc0 + r0:bc0 + r1, :m * w],
                                    ot[r0:r1, :m * w])
            bc0 += bsz

    nc = bacc.Bacc("TRN2", target_bir_lowering=False, debug=False,
                   num_devices=NCORES, enable_partition_id=False)
    with tile.TileContext(nc) as tc:
        build_kernel(tc)
    nc.compile()
    return nc


# --------------------------------------------------------------------------
# entry point
# --------------------------------------------------------------------------

def kernel(x_i, x_j, a, edge_index, num_nodes):
    x_i = np.asarray(x_i, dtype=np.float32)
    x_j = np.asarray(x_j, dtype=np.float32)
    a = np.asarray(a, dtype=np.float32)
    edge_index = np.asarray(edge_index)
    num_nodes = int(np.asarray(num_nodes))

    per_core, meta = _pack(x_i, x_j, a, edge_index, num_nodes)
    key = (meta["nchunk_core"], meta["runs"])

    if key not in _NC_CACHE:
        _NC_CACHE[key] = _build_nc(meta["nchunk_core"], meta["runs"])
    nc = _NC_CACHE[key]

    from concourse.bass_utils import run_bass_kernel_spmd
    in_maps = [dict(stream=per_core[i]["stream"], consts=meta["consts"])
               for i in range(NCORES)]
    res = run_bass_kernel_spmd(nc, in_maps, core_ids=list(range(NCORES)))

    full = np.concatenate([res.results[i]["out"].reshape(-1)
                           for i in range(NCORES)])
    return full[meta["slot_of_edge"]].astype(np.float32).reshape(-1, 1)
